# revision 4
# baseline (speedup 1.0000x reference)
"""Isolated single-head attention on 8 Trainium2 NeuronCores.

Problem: inp_emb (4, 4096, 1024) f32; Wq/Wk/Wv (1024, 1024) f32.
  Q = x @ Wq.T; K = x @ Wk.T; V = x @ Wv.T
  out = softmax(Q K^T / 32) @ V          (per batch)

Sharding: core c -> batch b = c//2, q-row half h = c%2 (2048 rows).
Keys are reordered per-core as [own-half rows, buddy-half rows] (softmax
is permutation-invariant over keys) so every address is static SPMD.

Algebraic restructure (removes the K projection and the duplicated
full-batch V projection entirely):
  A  = Wq^T @ Wk  (host, f32 -> bf16; shared across cores)
  S  = Q K^T = x_q A x^T          -> QA = x_q @ A   (own rows only)
  O  = P V  = (P x) Wv^T          -> PXT = x^T P^T, O = PX @ Wv^T
Per-core matmul work: QA 4.3 + S 17.2 + PX 17.2 + O-proj 4.3 GFLOP
(vs 55.8 GFLOP for the direct form with duplicated K/V).
fp8 was evaluated and rejected: e4m3 quantization of either operand of
any large matmul costs ~2.7% RMS -> rel err 2-5e-2, over the 2e-2 gate.

All matmuls bf16 with f32 PSUM accumulation. Layouts per core:
  xt  [1024 d, 4096 k] bf16  resident SBUF (scores lhsT + QA rhs)
  xr  [4096 k, 1024 d] bf16  resident SBUF (PXT lhsT)
  at  [1024 d, 1024 j] bf16  A            (QA lhsT; slot reused by EXP)
  wvt [1024 d, 1024 e] bf16  Wv^T         (O-proj rhs, resident)
Kernel phases (per core, fully unrolled):
  Warmup: 24 dummy matmuls on a memset tile while the head DMAs land,
    so the HAM clock-gate is at 2.4 GHz when real work starts.
  Head: A streams on the sync (SP) HWDGE queue in (j, dc-half) chunks,
    xt on the scalar (Activation) HWDGE queue in 512-col chunks, both
    in first-use order; xr/wvt/qat staging ride the gpsimd SWDGE ring.
  QA pass: QAT[j, q] -> DRAM scratch (SBUF is full); qc 0 stays in SBUF.
  Attention per 512-row q-block:
    ST[k, q] = xt-tile.T @ QAT-block   (PSUM f32, 8 accum MMs)
    EXP[k, q] = exp(ST/32) -> SBUF bf16 (no max subtraction: |s|<~9)
    acc[k, q] += EXP on DVE in f32 (partial key-tile sums)
    sums^T: per 128-q slice, ONE fp32 matmul acc-chunk.T @ ones -> [q,1]
      (q lands on partitions directly: no ones-row matmul, no transpose
      matmuls), reciprocal on DVE straight out of PSUM, all before the
      PX matmuls so the O-proj scaling never waits.
    PXT[d, q] = sum_k xr-tile.T @ EXP  (PSUM f32 -> SBUF bf16)
    O[q, e] = sum_d pxt-tile.T @ wvt, scaled by 1/sum on ScalarE,
    stored bf16 (host casts back to f32).
"""

import numpy as np
import ml_dtypes

D = 1024
S = 4096          # keys per batch
SQ = 2048         # q rows per core
QB = 512          # q-block
NQB = SQ // QB    # 4
NKT = S // 128    # 32 k tiles
ND = D // 128     # 8 chunks of d/j/e
SCALE = 1.0 / 32.0

_CACHE = {}
TRACE = False
LAST_RESULT = None


def _build():
    import concourse.bass as bass
    import concourse.bacc as bacc
    import concourse.mybir as mybir
    import concourse.tile as tile

    f32 = mybir.dt.float32
    bf16 = mybir.dt.bfloat16
    EXPF = mybir.ActivationFunctionType.Exp

    nc = bacc.Bacc(None)
    xt_d = nc.dram_tensor("xt", [D, S], bf16, kind="ExternalInput")
    xr_d = nc.dram_tensor("xr", [S, D], bf16, kind="ExternalInput")
    at_d = nc.dram_tensor("at", [D, D], bf16, kind="ExternalInput")
    wvt_d = nc.dram_tensor("wvt", [D, D], bf16, kind="ExternalInput")
    out_d = nc.dram_tensor("out", [SQ, D], bf16, kind="ExternalOutput")

    with tile.TileContext(nc) as tc:
        with (
            tc.tile_pool(name="xtp", bufs=1) as xtp,
            tc.tile_pool(name="xrp", bufs=1) as xrp,
            tc.tile_pool(name="wvp", bufs=1) as wvp,
            tc.tile_pool(name="big", bufs=2) as bigp,
            tc.tile_pool(name="qtp", bufs=2) as qtp,
            tc.tile_pool(name="pxp", bufs=1) as pxp,
            tc.tile_pool(name="stg", bufs=3) as stgp,
            tc.tile_pool(name="sml", bufs=2) as smlp,
            tc.tile_pool(name="acc", bufs=1) as accp,
            tc.tile_pool(name="cst", bufs=1) as cstp,
            tc.tile_pool(name="psQ", bufs=2, space="PSUM") as psQ,
            tc.tile_pool(name="psB", bufs=2, space="PSUM") as psB,
            tc.tile_pool(name="psX", bufs=2, space="PSUM") as psX,
            tc.tile_pool(name="psS", bufs=2, space="PSUM") as psS,
            tc.tile_pool(name="drm", bufs=1, space="DRAM") as drmp,
        ):
            xt_sb = xtp.tile([128, ND, S], bf16)      # xT[d, k]: 64KB/part
            xr_sb = xrp.tile([128, NKT, D], bf16)     # x[k, d]:  64KB/part
            wvt_sb = wvp.tile([128, ND, D], bf16)     # WvT[d, e]: 16KB/part
            qat_dram = drmp.tile([D, SQ], bf16)

            ones_sb = cstp.tile([128, 1], f32)
            dummy = cstp.tile([128, 512], bf16)
            nc.vector.memset(ones_sb[:], 1.0)
            nc.vector.memset(dummy[:], 0.0)

            # PE warmup: throwaway matmuls keep the PE busy through the
            # HAM SHORT window while the head DMAs land, so the first
            # real matmul runs at 2.4 GHz instead of 1.2 and the PE
            # never idles long enough to re-throttle. ~8 run cold
            # (~3.4us) flipping the clock gate; the rest bridge to the
            # arrival of the first QA operands (~15.5us).
            for w in range(30):
                ps_w = psB.tile([128, 512], f32, tag="psB")
                nc.tensor.matmul(
                    ps_w[:], dummy[:, 0:128], dummy[:], start=True, stop=True
                )

            # Head DMAs, first-use order, three queues in parallel
            # (measured: SWDGE ring ~112 GB/s, each HWDGE queue ~75
            # GB/s; DMA processing starts ~8.7us into the kernel):
            #   gpsimd: xt0 slabs 0:4, xt1..xt3 (QA rhs path), xt4..7,
            #           then qat staging + qt loads + out stores
            #   sync:   A j0..j3, then xr in 8 d-chunks (PX lhsT)
            #   scalar: xt0 slabs 4:8, A j4..j7, wvt, (EXPs later)
            a_sb = bigp.tile([128, ND, D], bf16, tag="big")
            at_re = at_d.rearrange("(c p) j -> p c j", p=128)
            xt_re = xt_d.rearrange("(j p) k -> p j k", p=128)
            nc.gpsimd.dma_start(
                out=xt_sb[:, 0:4, 0:512], in_=xt_re[:, 0:4, 0:512]
            )
            nc.scalar.dma_start(
                out=xt_sb[:, 4:8, 0:512], in_=xt_re[:, 4:8, 0:512]
            )
            for j in range(4):
                nc.sync.dma_start(
                    out=a_sb[:, :, j * 128:(j + 1) * 128],
                    in_=at_re[:, :, j * 128:(j + 1) * 128],
                )
            for kc in range(1, S // 512):
                nc.gpsimd.dma_start(
                    out=xt_sb[:, :, kc * 512:(kc + 1) * 512],
                    in_=xt_re[:, :, kc * 512:(kc + 1) * 512],
                )
            for j in range(4, ND):
                nc.scalar.dma_start(
                    out=a_sb[:, :, j * 128:(j + 1) * 128],
                    in_=at_re[:, :, j * 128:(j + 1) * 128],
                )

            # ---------------- QA pass: QAT[j, q] ----------------
            # qc 0 keeps its result in SBUF (used by the first attention
            # q-block with no DRAM roundtrip); qc 1..3 stage via DRAM.
            qt0 = qtp.tile([128, ND, QB], bf16, tag="qt", name="qt0")
            for qc in range(SQ // 512):
                for j in range(ND):
                    # Alternate the two (otherwise idle) PSUM pools for a
                    # 4-deep accumulator pipeline during the QA pass.
                    pool, ptag = (psQ, "psQ") if j % 2 == 0 else (psX, "psX")
                    ps = pool.tile([128, 512], f32, tag=ptag)
                    for dc in range(ND):
                        qa_mm = nc.tensor.matmul(
                            ps[:],
                            a_sb[:, dc, j * 128:(j + 1) * 128],
                            xt_sb[:, dc, qc * 512:(qc + 1) * 512],
                            start=(dc == 0), stop=(dc == ND - 1),
                        )
                    if qc == 0:
                        nc.vector.tensor_copy(qt0[:, j, :], ps[:])
                    else:
                        st = stgp.tile([128, 512], bf16, tag="stg")
                        nc.vector.tensor_copy(st[:], ps[:])
                        nc.gpsimd.dma_start(
                            out=qat_dram[j * 128:(j + 1) * 128,
                                         qc * 512:(qc + 1) * 512],
                            in_=st[:],
                        )

            # Bulk loads for the attention phase: xr rides the sync queue
            # behind A (first needed at PX of qb0, ~130us in), in PX's
            # d-chunk consumption order; wvt behind A on scalar.
            xr_re = xr_d.rearrange("(t p) d -> p t d", p=128)
            for dch in range(ND):
                nc.sync.dma_start(
                    out=xr_sb[:, :, dch * 128:(dch + 1) * 128],
                    in_=xr_re[:, :, dch * 128:(dch + 1) * 128],
                )
            nc.scalar.dma_start(
                out=wvt_sb[:], in_=wvt_d.rearrange("(c p) e -> p c e", p=128)
            )

            # ---------------- Attention ----------------
            for qb in range(NQB):
                q0 = qb * QB
                if qb == 0:
                    qt = qt0
                else:
                    qt = qtp.tile([128, ND, QB], bf16, tag="qt")
                    nc.gpsimd.dma_start(
                        out=qt[:],
                        in_=qat_dram[:].rearrange("(j p) q -> p j q", p=128)[
                            :, :, q0:q0 + QB
                        ],
                    )
                # EXP for this q-block lives in two 16-kt half tiles that
                # recycle the big pool's slots (a_sb is dead after QA).
                exp_h = [
                    bigp.tile([128, NKT // 2, QB], bf16, tag="big", name=f"exp{qb}_{i}")
                    for i in range(2)
                ]
                acc = accp.tile([128, QB], f32, tag="acc")
                for kt in range(NKT):
                    eh = exp_h[kt // 16]
                    ps_st = psB.tile([128, QB], f32, tag="psB")
                    for j in range(ND):
                        nc.tensor.matmul(
                            ps_st[:],
                            xt_sb[:, j, kt * 128:(kt + 1) * 128],
                            qt[:, j, :],
                            start=(j == 0), stop=(j == ND - 1),
                        )
                    nc.scalar.activation(eh[:, kt % 16, :], ps_st[:], EXPF, scale=SCALE)
                    # Partial key-tile sum on the (otherwise idle) DVE in f32
                    # — same precision as PSUM accumulation. The partition
                    # reduction then needs just ONE matmul per 128-q slice.
                    if kt == 0:
                        nc.vector.tensor_copy(acc[:], eh[:, 0, :])
                    else:
                        nc.vector.tensor_add(acc[:], acc[:], eh[:, kt % 16, :])

                # Transposed row-sums: acc-chunk as the stationary operand
                # puts q on the output partitions directly ([q,1] = chunk.T
                # @ ones). fp32 matmul at N=1 is ~LDW-cost only. Done before
                # PX so the reciprocals are long ready when O-proj needs
                # them.
                rcp = smlp.tile([128, NQB], f32, tag="rcp")
                for qs in range(QB // 128):
                    ps_s = psS.tile([128, 1], f32, tag="psS")
                    nc.tensor.matmul(
                        ps_s[:], acc[:, qs * 128:(qs + 1) * 128], ones_sb[:],
                        start=True, stop=True,
                    )
                    nc.vector.reciprocal(rcp[:, qs:qs + 1], ps_s[:])

                # PXT[d, q] = sum_k x[k, d] * EXP[k, q]
                pxt_sb = pxp.tile([128, ND, QB], bf16, tag="pxt")
                for dt in range(ND):
                    ps_px = psX.tile([128, QB], f32, tag="psX")
                    for kt in range(NKT):
                        nc.tensor.matmul(
                            ps_px[:],
                            xr_sb[:, kt, dt * 128:(dt + 1) * 128],
                            exp_h[kt // 16][:, kt % 16, :],
                            start=(kt == 0), stop=(kt == NKT - 1),
                        )
                    nc.vector.tensor_copy(pxt_sb[:, dt, :], ps_px[:])

                for qs in range(QB // 128):
                    for ec in range(2):
                        ps_o = psQ.tile([128, 512], f32, tag="psQ")
                        for dt in range(ND):
                            nc.tensor.matmul(
                                ps_o[:],
                                pxt_sb[:, dt, qs * 128:(qs + 1) * 128],
                                wvt_sb[:, dt, ec * 512:(ec + 1) * 512],
                                start=(dt == 0), stop=(dt == ND - 1),
                            )
                        o_sb = stgp.tile([128, 512], bf16, tag="stg")
                        nc.scalar.mul(o_sb[:], ps_o[:], rcp[:, qs:qs + 1])
                        nc.gpsimd.dma_start(
                            out=out_d[q0 + qs * 128:q0 + qs * 128 + 128,
                                      ec * 512:(ec + 1) * 512],
                            in_=o_sb[:],
                        )
    nc.compile()
    return nc


def kernel(inp_emb, Wq, Wk, Wv):
    global LAST_RESULT
    from concourse.bass_utils import run_bass_kernel_spmd

    bf = ml_dtypes.bfloat16
    x = np.asarray(inp_emb, dtype=np.float32)
    a = np.ascontiguousarray(
        np.asarray(Wq, np.float32).T @ np.asarray(Wk, np.float32)
    ).astype(bf)
    wvt = np.ascontiguousarray(np.asarray(Wv, np.float32).T).astype(bf)

    in_maps = []
    for c in range(8):
        b, h = divmod(c, 2)
        x_re = np.concatenate(
            [x[b, h * SQ:(h + 1) * SQ], x[b, (1 - h) * SQ:(2 - h) * SQ]], axis=0
        )
        xr = np.ascontiguousarray(x_re).astype(bf)          # (4096, 1024)
        xt = np.ascontiguousarray(x_re.T).astype(bf)        # (1024, 4096)
        in_maps.append({"xt": xt, "xr": xr, "at": a, "wvt": wvt})

    if "nc" not in _CACHE:
        _CACHE["nc"] = _build()
    nc = _CACHE["nc"]

    res = run_bass_kernel_spmd(nc, in_maps, list(range(8)), trace=TRACE)
    LAST_RESULT = res

    out = np.empty((4, S, D), dtype=np.float32)
    for c in range(8):
        b, h = divmod(c, 2)
        out[b, h * SQ:(h + 1) * SQ] = res.results[c]["out"].astype(np.float32)
    return out


# revision 6
# speedup vs baseline: 1.1978x; 1.1978x over previous
"""Isolated single-head attention on 8 Trainium2 NeuronCores.

Problem: inp_emb (4, 4096, 1024) f32; Wq/Wk/Wv (1024, 1024) f32.
  Q = x @ Wq.T; K = x @ Wk.T; V = x @ Wv.T
  out = softmax(Q K^T / 32) @ V          (per batch)

Sharding: core c -> batch b = c//2, q-row half h = c%2 (2048 rows).
Keys are reordered per-core as [own-half rows, buddy-half rows] (softmax
is permutation-invariant over keys) so every address is static SPMD.

Algebraic restructure (removes the K projection and the duplicated
full-batch V projection entirely):
  A  = Wq^T @ Wk  (host, f32 -> bf16; shared across cores)
  S  = Q K^T = x_q A x^T          -> QA = x_q @ A   (own rows only)
  O  = P V  = (P x) Wv^T          -> PXT = x^T P^T, O = PX @ Wv^T
Per-core matmul work: QA 4.3 + S 17.2 + PX 17.2 + O-proj 4.3 GFLOP
(vs 55.8 GFLOP for the direct form with duplicated K/V).
fp8 was evaluated and rejected: e4m3 quantization of either operand of
any large matmul costs ~2.7% RMS -> rel err 2-5e-2, over the 2e-2 gate.

All matmuls bf16 with f32 PSUM accumulation. Layouts per core:
  xt  [1024 d, 4096 k] bf16  resident SBUF (scores lhsT + QA rhs)
  xr  [4096 k, 1024 d] bf16  resident SBUF (PXT lhsT)
  at  [1024 d, 1024 j] bf16  A            (QA lhsT; slot reused by EXP)
  wvt [1024 d, 1024 e] bf16  Wv^T         (O-proj rhs, resident)
Kernel phases (per core, fully unrolled):
  Warmup: 24 dummy matmuls on a memset tile while the head DMAs land,
    so the HAM clock-gate is at 2.4 GHz when real work starts.
  Head: A streams on the sync (SP) HWDGE queue in (j, dc-half) chunks,
    xt on the scalar (Activation) HWDGE queue in 512-col chunks, both
    in first-use order; xr/wvt/qat staging ride the gpsimd SWDGE ring.
  QA pass: QAT[j, q] -> DRAM scratch (SBUF is full); qc 0 stays in SBUF.
  Attention per 512-row q-block:
    ST[k, q] = xt-tile.T @ QAT-block   (PSUM f32, 8 accum MMs)
    EXP[k, q] = exp(ST/32) -> SBUF bf16 (no max subtraction: |s|<~9)
    acc[k, q] += EXP on DVE in f32 (partial key-tile sums)
    sums^T: per 128-q slice, ONE fp32 matmul acc-chunk.T @ ones -> [q,1]
      (q lands on partitions directly: no ones-row matmul, no transpose
      matmuls), reciprocal on DVE straight out of PSUM, all before the
      PX matmuls so the O-proj scaling never waits.
    PXT[d, q] = sum_k xr-tile.T @ EXP  (PSUM f32 -> SBUF bf16)
    O[q, e] = sum_d pxt-tile.T @ wvt, scaled by 1/sum on ScalarE,
    stored bf16 (host casts back to f32).
"""

import numpy as np
import ml_dtypes

D = 1024
S = 4096          # keys per batch
SQ = 2048         # q rows per core
QB = 512          # q-block
NQB = SQ // QB    # 4
NKT = S // 128    # 32 k tiles
ND = D // 128     # 8 chunks of d/j/e
SCALE = 1.0 / 32.0

_CACHE = {}
TRACE = False
LAST_RESULT = None


def _build():
    import concourse.bass as bass
    import concourse.bacc as bacc
    import concourse.mybir as mybir
    import concourse.tile as tile

    f32 = mybir.dt.float32
    bf16 = mybir.dt.bfloat16
    EXPF = mybir.ActivationFunctionType.Exp

    nc = bacc.Bacc(None)
    xt_d = nc.dram_tensor("xt", [D, S], bf16, kind="ExternalInput")
    xr_d = nc.dram_tensor("xr", [S, D], bf16, kind="ExternalInput")
    at_d = nc.dram_tensor("at", [D, D], bf16, kind="ExternalInput")
    wvt_d = nc.dram_tensor("wvt", [D, D], bf16, kind="ExternalInput")
    out_d = nc.dram_tensor("out", [SQ, D], bf16, kind="ExternalOutput")

    with tile.TileContext(nc) as tc:
        with (
            tc.tile_pool(name="xtp", bufs=1) as xtp,
            tc.tile_pool(name="xrp", bufs=1) as xrp,
            tc.tile_pool(name="wvp", bufs=1) as wvp,
            tc.tile_pool(name="big", bufs=2) as bigp,
            tc.tile_pool(name="qtp", bufs=2) as qtp,
            tc.tile_pool(name="pxp", bufs=1) as pxp,
            tc.tile_pool(name="stg", bufs=3) as stgp,
            tc.tile_pool(name="sml", bufs=2) as smlp,
            tc.tile_pool(name="acc", bufs=1) as accp,
            tc.tile_pool(name="cst", bufs=1) as cstp,
            tc.tile_pool(name="psQ", bufs=2, space="PSUM") as psQ,
            tc.tile_pool(name="psB", bufs=2, space="PSUM") as psB,
            tc.tile_pool(name="psX", bufs=2, space="PSUM") as psX,
            tc.tile_pool(name="psS", bufs=2, space="PSUM") as psS,
            tc.tile_pool(name="drm", bufs=1, space="DRAM") as drmp,
        ):
            xt_sb = xtp.tile([128, ND, S], bf16)      # xT[d, k]: 64KB/part
            xr_sb = xrp.tile([128, NKT, D], bf16)     # x[k, d]:  64KB/part
            wvt_sb = wvp.tile([128, ND, D], bf16)     # WvT[d, e]: 16KB/part
            qat_dram = drmp.tile([D, SQ], bf16)

            ones_sb = cstp.tile([128, 1], f32)
            dummy = cstp.tile([128, 512], bf16)
            nc.vector.memset(ones_sb[:], 1.0)
            nc.vector.memset(dummy[:], 0.0)

            # PE warmup: throwaway matmuls keep the PE busy through the
            # HAM SHORT window while the head DMAs land, so the first
            # real matmul runs at 2.4 GHz instead of 1.2 and the PE
            # never idles long enough to re-throttle. ~8 run cold
            # (~3.4us) flipping the clock gate; the rest bridge to the
            # arrival of the first QA operands (~15.5us).
            for w in range(30):
                ps_w = psB.tile([128, 512], f32, tag="psB")
                nc.tensor.matmul(
                    ps_w[:], dummy[:, 0:128], dummy[:], start=True, stop=True
                )

            # Head DMAs, first-use order, three queues in parallel.
            # Chunks keep >=1KB contiguous per-partition segments (256B
            # strided chunks measured ~4x slower). DMA processing starts
            # ~8.7us into the kernel regardless (runtime init).
            #   gpsimd: xt0 slabs 0:4, xt1..xt7 (QA rhs + scores lhsT),
            #           then qat staging + qt loads + out stores
            #   sync:   A in 256-col j-chunks, then xr whole (PX lhsT)
            #   scalar: xt0 slabs 4:8, then wvt, (EXPs later)
            a_sb = bigp.tile([128, ND, D], bf16, tag="big")
            at_re = at_d.rearrange("(c p) j -> p c j", p=128)
            xt_re = xt_d.rearrange("(j p) k -> p j k", p=128)
            nc.gpsimd.dma_start(
                out=xt_sb[:, 0:4, 0:512], in_=xt_re[:, 0:4, 0:512]
            )
            nc.scalar.dma_start(
                out=xt_sb[:, 4:8, 0:512], in_=xt_re[:, 4:8, 0:512]
            )
            for jc in range(4):
                nc.sync.dma_start(
                    out=a_sb[:, :, jc * 256:(jc + 1) * 256],
                    in_=at_re[:, :, jc * 256:(jc + 1) * 256],
                )
            for kc in range(1, S // 512):
                nc.gpsimd.dma_start(
                    out=xt_sb[:, :, kc * 512:(kc + 1) * 512],
                    in_=xt_re[:, :, kc * 512:(kc + 1) * 512],
                )
            nc.scalar.dma_start(
                out=wvt_sb[:], in_=wvt_d.rearrange("(c p) e -> p c e", p=128)
            )

            # ---------------- QA pass: QAT[j, q] ----------------
            # qc 0 keeps its result in SBUF (used by the first attention
            # q-block with no DRAM roundtrip); qc 1..3 stage via DRAM.
            qt0 = qtp.tile([128, ND, QB], bf16, tag="qt", name="qt0")
            for qc in range(SQ // 512):
                for j in range(ND):
                    # Alternate the two (otherwise idle) PSUM pools for a
                    # 4-deep accumulator pipeline during the QA pass.
                    pool, ptag = (psQ, "psQ") if j % 2 == 0 else (psX, "psX")
                    ps = pool.tile([128, 512], f32, tag=ptag)
                    for dc in range(ND):
                        qa_mm = nc.tensor.matmul(
                            ps[:],
                            a_sb[:, dc, j * 128:(j + 1) * 128],
                            xt_sb[:, dc, qc * 512:(qc + 1) * 512],
                            start=(dc == 0), stop=(dc == ND - 1),
                        )
                    if qc == 0:
                        nc.vector.tensor_copy(qt0[:, j, :], ps[:])
                    else:
                        st = stgp.tile([128, 512], bf16, tag="stg")
                        nc.vector.tensor_copy(st[:], ps[:])
                        nc.gpsimd.dma_start(
                            out=qat_dram[j * 128:(j + 1) * 128,
                                         qc * 512:(qc + 1) * 512],
                            in_=st[:],
                        )

            # Bulk xr load for the attention phase: single contiguous DMA
            # behind A on the sync queue (starts ~26us, lands well before
            # PX of qb0 needs it at ~150us).
            nc.sync.dma_start(
                out=xr_sb[:], in_=xr_d.rearrange("(t p) d -> p t d", p=128)
            )

            # ---------------- Attention ----------------
            for qb in range(NQB):
                q0 = qb * QB
                if qb == 0:
                    qt = qt0
                else:
                    qt = qtp.tile([128, ND, QB], bf16, tag="qt")
                    nc.gpsimd.dma_start(
                        out=qt[:],
                        in_=qat_dram[:].rearrange("(j p) q -> p j q", p=128)[
                            :, :, q0:q0 + QB
                        ],
                    )
                # EXP for this q-block lives in two 16-kt half tiles that
                # recycle the big pool's slots (a_sb is dead after QA).
                exp_h = [
                    bigp.tile([128, NKT // 2, QB], bf16, tag="big", name=f"exp{qb}_{i}")
                    for i in range(2)
                ]
                acc = accp.tile([128, QB], f32, tag="acc")
                for kt in range(NKT):
                    eh = exp_h[kt // 16]
                    ps_st = psB.tile([128, QB], f32, tag="psB")
                    for j in range(ND):
                        nc.tensor.matmul(
                            ps_st[:],
                            xt_sb[:, j, kt * 128:(kt + 1) * 128],
                            qt[:, j, :],
                            start=(j == 0), stop=(j == ND - 1),
                        )
                    nc.scalar.activation(eh[:, kt % 16, :], ps_st[:], EXPF, scale=SCALE)
                    # Partial key-tile sum on the (otherwise idle) DVE in f32
                    # — same precision as PSUM accumulation. The partition
                    # reduction then needs just ONE matmul per 128-q slice.
                    if kt == 0:
                        nc.vector.tensor_copy(acc[:], eh[:, 0, :])
                    else:
                        nc.vector.tensor_add(acc[:], acc[:], eh[:, kt % 16, :])

                # Transposed row-sums: acc-chunk as the stationary operand
                # puts q on the output partitions directly ([q,1] = chunk.T
                # @ ones). fp32 matmul at N=1 is ~LDW-cost only. Done before
                # PX so the reciprocals are long ready when O-proj needs
                # them.
                rcp = smlp.tile([128, NQB], f32, tag="rcp")
                for qs in range(QB // 128):
                    ps_s = psS.tile([128, 1], f32, tag="psS")
                    nc.tensor.matmul(
                        ps_s[:], acc[:, qs * 128:(qs + 1) * 128], ones_sb[:],
                        start=True, stop=True,
                    )
                    nc.vector.reciprocal(rcp[:, qs:qs + 1], ps_s[:])

                # PXT[d, q] = sum_k x[k, d] * EXP[k, q]
                pxt_sb = pxp.tile([128, ND, QB], bf16, tag="pxt")
                for dt in range(ND):
                    ps_px = psX.tile([128, QB], f32, tag="psX")
                    for kt in range(NKT):
                        nc.tensor.matmul(
                            ps_px[:],
                            xr_sb[:, kt, dt * 128:(dt + 1) * 128],
                            exp_h[kt // 16][:, kt % 16, :],
                            start=(kt == 0), stop=(kt == NKT - 1),
                        )
                    nc.vector.tensor_copy(pxt_sb[:, dt, :], ps_px[:])

                for qs in range(QB // 128):
                    for ec in range(2):
                        ps_o = psQ.tile([128, 512], f32, tag="psQ")
                        for dt in range(ND):
                            nc.tensor.matmul(
                                ps_o[:],
                                pxt_sb[:, dt, qs * 128:(qs + 1) * 128],
                                wvt_sb[:, dt, ec * 512:(ec + 1) * 512],
                                start=(dt == 0), stop=(dt == ND - 1),
                            )
                        o_sb = stgp.tile([128, 512], bf16, tag="stg")
                        nc.scalar.mul(o_sb[:], ps_o[:], rcp[:, qs:qs + 1])
                        nc.gpsimd.dma_start(
                            out=out_d[q0 + qs * 128:q0 + qs * 128 + 128,
                                      ec * 512:(ec + 1) * 512],
                            in_=o_sb[:],
                        )
    nc.compile()
    return nc


def kernel(inp_emb, Wq, Wk, Wv):
    global LAST_RESULT
    from concourse.bass_utils import run_bass_kernel_spmd

    bf = ml_dtypes.bfloat16
    x = np.asarray(inp_emb, dtype=np.float32)
    a = np.ascontiguousarray(
        np.asarray(Wq, np.float32).T @ np.asarray(Wk, np.float32)
    ).astype(bf)
    wvt = np.ascontiguousarray(np.asarray(Wv, np.float32).T).astype(bf)

    in_maps = []
    for c in range(8):
        b, h = divmod(c, 2)
        x_re = np.concatenate(
            [x[b, h * SQ:(h + 1) * SQ], x[b, (1 - h) * SQ:(2 - h) * SQ]], axis=0
        )
        xr = np.ascontiguousarray(x_re).astype(bf)          # (4096, 1024)
        xt = np.ascontiguousarray(x_re.T).astype(bf)        # (1024, 4096)
        in_maps.append({"xt": xt, "xr": xr, "at": a, "wvt": wvt})

    if "nc" not in _CACHE:
        _CACHE["nc"] = _build()
    nc = _CACHE["nc"]

    res = run_bass_kernel_spmd(nc, in_maps, list(range(8)), trace=TRACE)
    LAST_RESULT = res

    out = np.empty((4, S, D), dtype=np.float32)
    for c in range(8):
        b, h = divmod(c, 2)
        out[b, h * SQ:(h + 1) * SQ] = res.results[c]["out"].astype(np.float32)
    return out


# revision 8
# speedup vs baseline: 1.2304x; 1.0273x over previous
"""Isolated single-head attention on 8 Trainium2 NeuronCores.

Problem: inp_emb (4, 4096, 1024) f32; Wq/Wk/Wv (1024, 1024) f32.
  Q = x @ Wq.T; K = x @ Wk.T; V = x @ Wv.T
  out = softmax(Q K^T / 32) @ V          (per batch)

Sharding: core c -> batch b = c//2, q-row half h = c%2 (2048 rows).
Keys are reordered per-core as [own-half rows, buddy-half rows] (softmax
is permutation-invariant over keys) so every address is static SPMD.

Algebraic restructure (removes the K projection and the duplicated
full-batch V projection entirely):
  A  = Wq^T @ Wk  (host, f32 -> bf16; shared across cores)
  S  = Q K^T = x_q A x^T          -> QA = x_q @ A   (own rows only)
  O  = P V  = (P x) Wv^T          -> PXT = x^T P^T, O = PX @ Wv^T
Per-core matmul work: QA 4.3 + S 17.2 + PX 17.2 + O-proj 4.3 GFLOP
(vs 55.8 GFLOP for the direct form with duplicated K/V).
fp8 was evaluated and rejected: e4m3 quantization of either operand of
any large matmul costs ~2.7% RMS -> rel err 2-5e-2, over the 2e-2 gate.

All matmuls bf16 with f32 PSUM accumulation. Layouts per core:
  xt  [1024 d, 4096 k] bf16  resident SBUF (scores lhsT + QA rhs)
  xr  [4096 k, 1024 d] bf16  resident SBUF (PXT lhsT)
  at  [1024 d, 1024 j] bf16  A            (QA lhsT; slot reused by EXP)
  wvt [1024 d, 1024 e] bf16  Wv^T         (O-proj rhs, resident)
Kernel phases (per core, fully unrolled):
  Warmup: 24 dummy matmuls on a memset tile while the head DMAs land,
    so the HAM clock-gate is at 2.4 GHz when real work starts.
  Head: A streams on the sync (SP) HWDGE queue in (j, dc-half) chunks,
    xt on the scalar (Activation) HWDGE queue in 512-col chunks, both
    in first-use order; xr/wvt/qat staging ride the gpsimd SWDGE ring.
  QA pass: QAT[j, q] -> DRAM scratch (SBUF is full); qc 0 stays in SBUF.
  Attention per 512-row q-block:
    ST[k, q] = xt-tile.T @ QAT-block   (PSUM f32, 8 accum MMs)
    EXP[k, q] = exp(ST/32) -> SBUF bf16 (no max subtraction: |s|<~9)
    acc[k, q] += EXP on DVE in f32 (partial key-tile sums)
    sums^T: per 128-q slice, ONE fp32 matmul acc-chunk.T @ ones -> [q,1]
      (q lands on partitions directly: no ones-row matmul, no transpose
      matmuls), reciprocal on DVE straight out of PSUM, all before the
      PX matmuls so the O-proj scaling never waits.
    PXT[d, q] = sum_k xr-tile.T @ EXP  (PSUM f32 -> SBUF bf16)
    O[q, e] = sum_d pxt-tile.T @ wvt, scaled by 1/sum on ScalarE,
    stored bf16 (host casts back to f32).
"""

import numpy as np
import ml_dtypes

D = 1024
S = 4096          # keys per batch
SQ = 2048         # q rows per core
QB = 512          # q-block
NQB = SQ // QB    # 4
NKT = S // 128    # 32 k tiles
ND = D // 128     # 8 chunks of d/j/e
SCALE = 1.0 / 32.0

_CACHE = {}
TRACE = False
LAST_RESULT = None


def _build():
    import concourse.bass as bass
    import concourse.bacc as bacc
    import concourse.mybir as mybir
    import concourse.tile as tile

    f32 = mybir.dt.float32
    bf16 = mybir.dt.bfloat16
    EXPF = mybir.ActivationFunctionType.Exp

    nc = bacc.Bacc(None)
    xt_d = nc.dram_tensor("xt", [D, S], bf16, kind="ExternalInput")
    xr_d = nc.dram_tensor("xr", [S, D], bf16, kind="ExternalInput")
    at_d = nc.dram_tensor("at", [D, D], bf16, kind="ExternalInput")
    wvt_d = nc.dram_tensor("wvt", [D, D], bf16, kind="ExternalInput")
    out_d = nc.dram_tensor("out", [SQ, D], bf16, kind="ExternalOutput")

    with tile.TileContext(nc) as tc:
        with (
            tc.tile_pool(name="xtp", bufs=1) as xtp,
            tc.tile_pool(name="xrp", bufs=1) as xrp,
            tc.tile_pool(name="wvp", bufs=1) as wvp,
            tc.tile_pool(name="big", bufs=2) as bigp,
            tc.tile_pool(name="qtp", bufs=2) as qtp,
            tc.tile_pool(name="pxp", bufs=1) as pxp,
            tc.tile_pool(name="stg", bufs=3) as stgp,
            tc.tile_pool(name="sml", bufs=2) as smlp,
            tc.tile_pool(name="acc", bufs=1) as accp,
            tc.tile_pool(name="cst", bufs=1) as cstp,
            tc.tile_pool(name="psQ", bufs=2, space="PSUM") as psQ,
            tc.tile_pool(name="psB", bufs=2, space="PSUM") as psB,
            tc.tile_pool(name="psX", bufs=2, space="PSUM") as psX,
            tc.tile_pool(name="psS", bufs=2, space="PSUM") as psS,
            tc.tile_pool(name="drm", bufs=1, space="DRAM") as drmp,
        ):
            xt_sb = xtp.tile([128, ND, S], bf16)      # xT[d, k]: 64KB/part
            xr_sb = xrp.tile([128, NKT, D], bf16)     # x[k, d]:  64KB/part
            wvt_sb = wvp.tile([128, ND, D], bf16)     # WvT[d, e]: 16KB/part
            qat_dram = drmp.tile([D, SQ], bf16)

            ones_sb = cstp.tile([128, 1], f32)
            dummy = cstp.tile([128, 512], bf16)
            nc.vector.memset(ones_sb[:], 1.0)
            nc.vector.memset(dummy[:], 0.0)

            # PE warmup: throwaway matmuls keep the PE busy from ~8us
            # (engine start) until the first QA operands land (~22us).
            # ~8 run at the cold 1.2 GHz clock (3.4us, exactly the HAM
            # SHORT window) flipping the clock gate to 2.4 GHz; the
            # remaining 40 bridge to ~20.2us so the idle gap before the
            # first real matmul stays under the ~3.4us MID window and
            # the QA pass starts at full clock.
            for w in range(48):
                ps_w = psB.tile([128, 512], f32, tag="psB")
                nc.tensor.matmul(
                    ps_w[:], dummy[:, 0:128], dummy[:], start=True, stop=True
                )

            # Head DMAs, first-use order. Only the gpsimd SWDGE ring
            # reliably starts processing at ~8.7us (measured ~112 GB/s);
            # HWDGE queues start later, so they carry only what is
            # needed later: A's j4..7 half + wvt on scalar, xr on sync
            # (gated past QA below). Chunks keep >=512B contiguous
            # per-partition segments (256B chunks measured ~4x slower).
            a_sb = bigp.tile([128, ND, D], bf16, tag="big")
            at_re = at_d.rearrange("(c p) j -> p c j", p=128)
            xt_re = xt_d.rearrange("(j p) k -> p j k", p=128)
            nc.gpsimd.dma_start(out=xt_sb[:, :, 0:512], in_=xt_re[:, :, 0:512])
            for jq in range(2):
                nc.gpsimd.dma_start(
                    out=a_sb[:, :, jq * 256:(jq + 1) * 256],
                    in_=at_re[:, :, jq * 256:(jq + 1) * 256],
                )
            nc.scalar.dma_start(
                out=a_sb[:, :, 512:1024], in_=at_re[:, :, 512:1024]
            )
            for kc in range(1, S // 512):
                nc.gpsimd.dma_start(
                    out=xt_sb[:, :, kc * 512:(kc + 1) * 512],
                    in_=xt_re[:, :, kc * 512:(kc + 1) * 512],
                )
            nc.scalar.dma_start(
                out=wvt_sb[:], in_=wvt_d.rearrange("(c p) e -> p c e", p=128)
            )

            # ---------------- QA pass: QAT[j, q] ----------------
            # qc 0 keeps its result in SBUF (used by the first attention
            # q-block with no DRAM roundtrip); qc 1..3 stage via DRAM.
            qt0 = qtp.tile([128, ND, QB], bf16, tag="qt", name="qt0")
            for qc in range(SQ // 512):
                for j in range(ND):
                    # Alternate the two (otherwise idle) PSUM pools for a
                    # 4-deep accumulator pipeline during the QA pass.
                    pool, ptag = (psQ, "psQ") if j % 2 == 0 else (psX, "psX")
                    ps = pool.tile([128, 512], f32, tag=ptag)
                    for dc in range(ND):
                        qa_mm = nc.tensor.matmul(
                            ps[:],
                            a_sb[:, dc, j * 128:(j + 1) * 128],
                            xt_sb[:, dc, qc * 512:(qc + 1) * 512],
                            start=(dc == 0), stop=(dc == ND - 1),
                        )
                    if qc == 0:
                        nc.vector.tensor_copy(qt0[:, j, :], ps[:])
                    else:
                        st = stgp.tile([128, 512], bf16, tag="stg")
                        nc.vector.tensor_copy(st[:], ps[:])
                        nc.gpsimd.dma_start(
                            out=qat_dram[j * 128:(j + 1) * 128,
                                         qc * 512:(qc + 1) * 512],
                            in_=st[:],
                        )

            # Bulk xr load on the sync queue, gated behind the QA pass so
            # it doesn't fight the head loads and qat stores for HBM
            # bandwidth (first needed by PX of qb0, ~150us in).
            xr_dma = nc.sync.dma_start(
                out=xr_sb[:], in_=xr_d.rearrange("(t p) d -> p t d", p=128)
            )
            bass._add_dep_helper(
                xr_dma.ins, qa_mm.ins, reason="delay xr load past the QA pass"
            )

            # ---------------- Attention ----------------
            for qb in range(NQB):
                q0 = qb * QB
                if qb == 0:
                    qt = qt0
                else:
                    qt = qtp.tile([128, ND, QB], bf16, tag="qt")
                    nc.gpsimd.dma_start(
                        out=qt[:],
                        in_=qat_dram[:].rearrange("(j p) q -> p j q", p=128)[
                            :, :, q0:q0 + QB
                        ],
                    )
                # EXP for this q-block lives in two 16-kt half tiles that
                # recycle the big pool's slots (a_sb is dead after QA).
                exp_h = [
                    bigp.tile([128, NKT // 2, QB], bf16, tag="big", name=f"exp{qb}_{i}")
                    for i in range(2)
                ]
                acc = accp.tile([128, QB], f32, tag="acc")
                for kt in range(NKT):
                    eh = exp_h[kt // 16]
                    ps_st = psB.tile([128, QB], f32, tag="psB")
                    for j in range(ND):
                        nc.tensor.matmul(
                            ps_st[:],
                            xt_sb[:, j, kt * 128:(kt + 1) * 128],
                            qt[:, j, :],
                            start=(j == 0), stop=(j == ND - 1),
                        )
                    nc.scalar.activation(eh[:, kt % 16, :], ps_st[:], EXPF, scale=SCALE)
                    # Partial key-tile sum on the (otherwise idle) DVE in f32
                    # — same precision as PSUM accumulation. The partition
                    # reduction then needs just ONE matmul per 128-q slice.
                    if kt == 0:
                        nc.vector.tensor_copy(acc[:], eh[:, 0, :])
                    else:
                        nc.vector.tensor_add(acc[:], acc[:], eh[:, kt % 16, :])

                # Transposed row-sums: acc-chunk as the stationary operand
                # puts q on the output partitions directly ([q,1] = chunk.T
                # @ ones). fp32 matmul at N=1 is ~LDW-cost only. Done before
                # PX so the reciprocals are long ready when O-proj needs
                # them.
                rcp = smlp.tile([128, NQB], f32, tag="rcp")
                for qs in range(QB // 128):
                    ps_s = psS.tile([128, 1], f32, tag="psS")
                    nc.tensor.matmul(
                        ps_s[:], acc[:, qs * 128:(qs + 1) * 128], ones_sb[:],
                        start=True, stop=True,
                    )
                    nc.vector.reciprocal(rcp[:, qs:qs + 1], ps_s[:])

                # PXT[d, q] = sum_k x[k, d] * EXP[k, q]
                pxt_sb = pxp.tile([128, ND, QB], bf16, tag="pxt")
                for dt in range(ND):
                    ps_px = psX.tile([128, QB], f32, tag="psX")
                    for kt in range(NKT):
                        nc.tensor.matmul(
                            ps_px[:],
                            xr_sb[:, kt, dt * 128:(dt + 1) * 128],
                            exp_h[kt // 16][:, kt % 16, :],
                            start=(kt == 0), stop=(kt == NKT - 1),
                        )
                    nc.vector.tensor_copy(pxt_sb[:, dt, :], ps_px[:])

                for qs in range(QB // 128):
                    for ec in range(2):
                        ps_o = psQ.tile([128, 512], f32, tag="psQ")
                        for dt in range(ND):
                            nc.tensor.matmul(
                                ps_o[:],
                                pxt_sb[:, dt, qs * 128:(qs + 1) * 128],
                                wvt_sb[:, dt, ec * 512:(ec + 1) * 512],
                                start=(dt == 0), stop=(dt == ND - 1),
                            )
                        o_sb = stgp.tile([128, 512], bf16, tag="stg")
                        nc.scalar.mul(o_sb[:], ps_o[:], rcp[:, qs:qs + 1])
                        nc.gpsimd.dma_start(
                            out=out_d[q0 + qs * 128:q0 + qs * 128 + 128,
                                      ec * 512:(ec + 1) * 512],
                            in_=o_sb[:],
                        )
    nc.compile()
    return nc


def kernel(inp_emb, Wq, Wk, Wv):
    global LAST_RESULT
    from concourse.bass_utils import run_bass_kernel_spmd

    bf = ml_dtypes.bfloat16
    x = np.asarray(inp_emb, dtype=np.float32)
    a = np.ascontiguousarray(
        np.asarray(Wq, np.float32).T @ np.asarray(Wk, np.float32)
    ).astype(bf)
    wvt = np.ascontiguousarray(np.asarray(Wv, np.float32).T).astype(bf)

    in_maps = []
    for c in range(8):
        b, h = divmod(c, 2)
        x_re = np.concatenate(
            [x[b, h * SQ:(h + 1) * SQ], x[b, (1 - h) * SQ:(2 - h) * SQ]], axis=0
        )
        xr = np.ascontiguousarray(x_re).astype(bf)          # (4096, 1024)
        xt = np.ascontiguousarray(x_re.T).astype(bf)        # (1024, 4096)
        in_maps.append({"xt": xt, "xr": xr, "at": a, "wvt": wvt})

    if "nc" not in _CACHE:
        _CACHE["nc"] = _build()
    nc = _CACHE["nc"]

    res = run_bass_kernel_spmd(nc, in_maps, list(range(8)), trace=TRACE)
    LAST_RESULT = res

    out = np.empty((4, S, D), dtype=np.float32)
    for c in range(8):
        b, h = divmod(c, 2)
        out[b, h * SQ:(h + 1) * SQ] = res.results[c]["out"].astype(np.float32)
    return out


# revision 22
# speedup vs baseline: 1.2781x; 1.0387x over previous
"""Isolated single-head attention on 8 Trainium2 NeuronCores.

Problem: inp_emb (4, 4096, 1024) f32; Wq/Wk/Wv (1024, 1024) f32.
  Q = x @ Wq.T; K = x @ Wk.T; V = x @ Wv.T
  out = softmax(Q K^T / 32) @ V          (per batch)

Sharding: core c -> batch b = c//2, q-row half h = c%2 (2048 rows).
Keys are reordered per-core as [own-half rows, buddy-half rows] (softmax
is permutation-invariant over keys) so every address is static SPMD.

Algebraic restructure (removes the K projection and the duplicated
full-batch V projection entirely):
  A  = Wq^T @ Wk  (host, f32 -> bf16; shared across cores)
  S  = Q K^T = x_q A x^T          -> QA = x_q @ A   (own rows only)
  O  = P V  = (P x) Wv^T          -> PXT = x^T P^T, O = PX @ Wv^T
Per-core matmul work: QA 4.3 + S 17.2 + PX 17.2 + O-proj 4.3 GFLOP
(vs 55.8 GFLOP for the direct form with duplicated K/V).

Precision budget (gate: rel < 2e-2; all-bf16 lands at 4.1e-3): full
e4m3 anywhere costs 2-5e-2 (either operand of any big matmul adds
~2.7% RMS), but fp8 on a FRACTION of the PX contraction scales as
sqrt(n/32): n=8 trailing k-slabs sim at 1.54e-2. Those 8 slabs ride 4
DoubleRow MMs (2 slabs per MM at bf16 per-MM cost, measured 216ns) —
a 12.5% cut of the PX phase for ~0.9e-2 of error. Scores/QA stay bf16
(their errors feed the softmax exponent and cost 2x more).

Matmuls bf16 + 4 fp8-DR per PX group, f32 PSUM accumulation. Layouts:
  xt  [1024 d, 4096 k] bf16  resident SBUF (scores lhsT + QA rhs)
  xr  [3072 k, 1024 d] bf16  resident SBUF (PXT lhsT, kt 0..23)
  xr8 [1024 k, 1024 d] fp8   resident SBUF (PXT DR lhsT, kt 24..31)
  at  [1024 d, 1024 j] bf16  A            (QA lhsT; slot reused by EXP)
  wvt [1024 d, 1024 e] bf16  Wv^T         (O-proj rhs, resident)
EXP is exp(s/32 - 4): the shift keeps fp8 EXP under e4m3's max 240
(max |score| 8.8) and cancels in the normalization.
Kernel phases (per core, fully unrolled):
  Warmup: 24 dummy matmuls on a memset tile while the head DMAs land,
    so the HAM clock-gate is at 2.4 GHz when real work starts.
  Head: A streams on the sync (SP) HWDGE queue in (j, dc-half) chunks,
    xt on the scalar (Activation) HWDGE queue in 512-col chunks, both
    in first-use order; xr/wvt/qat staging ride the gpsimd SWDGE ring.
  QA pass: QAT[j, q] -> DRAM scratch (SBUF is full); qc 0 stays in SBUF.
  Attention per 512-row q-block:
    ST[k, q] = xt-tile.T @ QAT-block   (PSUM f32, 8 accum MMs)
    EXP[k, q] = exp(ST/32) -> SBUF bf16 (no max subtraction: |s|<~9)
    acc[k, q] += EXP on DVE in f32 (partial key-tile sums)
    sums^T: per 128-q slice, ONE fp32 matmul acc-chunk.T @ ones -> [q,1]
      (q lands on partitions directly: no ones-row matmul, no transpose
      matmuls), reciprocal on DVE straight out of PSUM, all before the
      PX matmuls so the O-proj scaling never waits.
    PXT[d, q] = sum_k xr-tile.T @ EXP  (PSUM f32 -> SBUF bf16)
    O[q, e] = sum_d pxt-tile.T @ wvt, scaled by 1/sum on ScalarE,
    stored bf16 (host casts back to f32).
"""

import numpy as np
import ml_dtypes

D = 1024
S = 4096          # keys per batch
SQ = 2048         # q rows per core
QB = 512          # q-block
NQB = SQ // QB    # 4
NKT = S // 128    # 32 k tiles
ND = D // 128     # 8 chunks of d/j/e
SCALE = 1.0 / 32.0
NK8 = 8           # trailing k-slabs computed in fp8 DoubleRow (PX only)
NKB = NKT - NK8   # leading k-slabs kept bf16
ESHIFT = -4.0     # exp(s/32 - 4): keeps fp8 EXP under e4m3 max 240
                  # (max |score| is 8.8); cancels in the normalization

_CACHE = {}
TRACE = False
LAST_RESULT = None


def _build():
    import concourse.bass as bass
    import concourse.bacc as bacc
    import concourse.mybir as mybir
    import concourse.tile as tile

    f32 = mybir.dt.float32
    bf16 = mybir.dt.bfloat16
    fp8 = mybir.dt.float8e4
    EXPF = mybir.ActivationFunctionType.Exp
    DR = mybir.MatmulPerfMode.DoubleRow

    nc = bacc.Bacc(None)
    xt_d = nc.dram_tensor("xt", [D, S], bf16, kind="ExternalInput")
    xr_d = nc.dram_tensor("xr", [128 * NKB, D], bf16, kind="ExternalInput")
    xr8_d = nc.dram_tensor("xr8", [128 * NK8, D], fp8, kind="ExternalInput")
    at_d = nc.dram_tensor("at", [D, D], bf16, kind="ExternalInput")
    wvt_d = nc.dram_tensor("wvt", [D, D], bf16, kind="ExternalInput")
    out_d = nc.dram_tensor("out", [SQ, D], bf16, kind="ExternalOutput")

    from contextlib import ExitStack

    with tile.TileContext(nc) as tc, ExitStack() as es:
        if True:  # keep the original body indentation
            pool = lambda *a, **k: es.enter_context(tc.tile_pool(*a, **k))
            xtp = pool(name="xtp", bufs=1)
            xrp = pool(name="xrp", bufs=1)
            xr8p = pool(name="xr8p", bufs=1)
            e8p = pool(name="e8p", bufs=2)
            wvp = pool(name="wvp", bufs=1)
            bigp = pool(name="big", bufs=2)
            qtp = pool(name="qtp", bufs=2)
            pxp = pool(name="pxp", bufs=1)
            stgp = pool(name="stg", bufs=3)
            smlp = pool(name="sml", bufs=2)
            accp = pool(name="acc", bufs=1)
            cstp = pool(name="cst", bufs=1)
            psQ = pool(name="psQ", bufs=2, space="PSUM")
            psB = pool(name="psB", bufs=2, space="PSUM")
            psX = pool(name="psX", bufs=2, space="PSUM")
            psS = pool(name="psS", bufs=2, space="PSUM")
            drmp = pool(name="drm", bufs=1, space="DRAM")
            xt_sb = xtp.tile([128, ND, S], bf16)      # xT[d, k]: 64KB/part
            xr_sb = xrp.tile([128, NKB, D], bf16)     # x[k<24t, d]: 48KB/part
            xr8_sb = xr8p.tile([128, NK8, D], fp8)    # x[k 24..31t, d]: 8KB
            wvt_sb = wvp.tile([128, ND, D], bf16)     # WvT[d, e]: 16KB/part
            qat_dram = drmp.tile([D, SQ], bf16)

            ones_sb = cstp.tile([128, 1], f32)
            eshift_sb = cstp.tile([128, 1], f32)
            dummy = cstp.tile([128, 512], bf16)
            nc.vector.memset(ones_sb[:], 1.0)
            nc.vector.memset(eshift_sb[:], ESHIFT)
            nc.vector.memset(dummy[:], 0.0)

            # PE warmup: throwaway matmuls keep the PE busy from ~8us
            # (engine start) until the first QA operands land (~22us).
            # ~8 run at the cold 1.2 GHz clock (3.4us, exactly the HAM
            # SHORT window) flipping the clock gate to 2.4 GHz; the
            # remaining 40 bridge to ~20.2us so the idle gap before the
            # first real matmul stays under the ~3.4us MID window and
            # the QA pass starts at full clock.
            for w in range(48):
                ps_w = psB.tile([128, 512], f32, tag="psB")
                nc.tensor.matmul(
                    ps_w[:], dummy[:, 0:128], dummy[:], start=True, stop=True
                )

            # Head DMAs, first-use order. Only the gpsimd SWDGE ring
            # reliably starts processing at ~8.7us (measured ~112 GB/s);
            # HWDGE queues start later, so they carry only what is
            # needed later: A's j4..7 half + wvt on scalar, xr on sync
            # (gated past QA below). Chunks keep >=512B contiguous
            # per-partition segments (256B chunks measured ~4x slower).
            a_sb = bigp.tile([128, ND, D], bf16, tag="big")
            at_re = at_d.rearrange("(c p) j -> p c j", p=128)
            xt_re = xt_d.rearrange("(j p) k -> p j k", p=128)
            nc.gpsimd.dma_start(out=xt_sb[:, :, 0:512], in_=xt_re[:, :, 0:512])
            for jq in range(2):
                nc.gpsimd.dma_start(
                    out=a_sb[:, :, jq * 256:(jq + 1) * 256],
                    in_=at_re[:, :, jq * 256:(jq + 1) * 256],
                )
            nc.scalar.dma_start(
                out=a_sb[:, :, 512:1024], in_=at_re[:, :, 512:1024]
            )
            for kc in range(1, S // 512):
                nc.gpsimd.dma_start(
                    out=xt_sb[:, :, kc * 512:(kc + 1) * 512],
                    in_=xt_re[:, :, kc * 512:(kc + 1) * 512],
                )
            nc.scalar.dma_start(
                out=wvt_sb[:], in_=wvt_d.rearrange("(c p) e -> p c e", p=128)
            )

            # ---------------- QA pass: QAT[j, q] ----------------
            # qc 0 keeps its result in SBUF (used by the first attention
            # q-block with no DRAM roundtrip); qc 1..3 stage via DRAM.
            qt0 = qtp.tile([128, ND, QB], bf16, tag="qt", name="qt0")
            for qc in range(SQ // 512):
                for j in range(ND):
                    # Alternate the two (otherwise idle) PSUM pools for a
                    # 4-deep accumulator pipeline during the QA pass.
                    pool, ptag = (psQ, "psQ") if j % 2 == 0 else (psX, "psX")
                    ps = pool.tile([128, 512], f32, tag=ptag)
                    for dc in range(ND):
                        qa_mm = nc.tensor.matmul(
                            ps[:],
                            a_sb[:, dc, j * 128:(j + 1) * 128],
                            xt_sb[:, dc, qc * 512:(qc + 1) * 512],
                            start=(dc == 0), stop=(dc == ND - 1),
                        )
                    if qc == 0:
                        nc.vector.tensor_copy(qt0[:, j, :], ps[:])
                    else:
                        st = stgp.tile([128, 512], bf16, tag="stg")
                        nc.vector.tensor_copy(st[:], ps[:])
                        nc.gpsimd.dma_start(
                            out=qat_dram[j * 128:(j + 1) * 128,
                                         qc * 512:(qc + 1) * 512],
                            in_=st[:],
                        )

            # Bulk xr/xr8 loads on the sync queue, gated behind the QA
            # pass so they don't fight the head loads and qat stores for
            # HBM bandwidth (first needed by PX of qb0, ~150us in).
            xr_dma = nc.sync.dma_start(
                out=xr_sb[:], in_=xr_d.rearrange("(t p) d -> p t d", p=128)
            )
            bass._add_dep_helper(
                xr_dma.ins, qa_mm.ins, reason="delay xr load past the QA pass"
            )
            nc.sync.dma_start(
                out=xr8_sb[:], in_=xr8_d.rearrange("(t p) d -> p t d", p=128)
            )

            # ---------------- Attention ----------------
            for qb in range(NQB):
                q0 = qb * QB
                if qb == 0:
                    qt = qt0
                else:
                    qt = qtp.tile([128, ND, QB], bf16, tag="qt")
                    nc.gpsimd.dma_start(
                        out=qt[:],
                        in_=qat_dram[:].rearrange("(j p) q -> p j q", p=128)[
                            :, :, q0:q0 + QB
                        ],
                    )
                # EXP for this q-block: kt 0..15 and 16..23 in bf16 tiles
                # recycling the big pool's slots (a_sb is dead after QA),
                # kt 24..31 in an fp8 tile feeding the DoubleRow PX tail.
                exp_h = [
                    bigp.tile([128, 16, QB], bf16, tag="big", name=f"exp{qb}_0"),
                    bigp.tile([128, NKB - 16, QB], bf16, tag="big", name=f"exp{qb}_1"),
                ]
                exp8 = e8p.tile([128, NK8, QB], fp8, tag="e8")
                eslot = (
                    [exp_h[0][:, k, :] for k in range(16)]
                    + [exp_h[1][:, k, :] for k in range(NKB - 16)]
                    + [exp8[:, k, :] for k in range(NK8)]
                )
                acc = accp.tile([128, QB], f32, tag="acc")
                for kt in range(NKT):
                    ps_st = psB.tile([128, QB], f32, tag="psB")
                    for j in range(ND):
                        nc.tensor.matmul(
                            ps_st[:],
                            xt_sb[:, j, kt * 128:(kt + 1) * 128],
                            qt[:, j, :],
                            start=(j == 0), stop=(j == ND - 1),
                        )
                    nc.scalar.activation(
                        eslot[kt], ps_st[:], EXPF, scale=SCALE, bias=eshift_sb[:]
                    )
                    # Partial key-tile sum on the (otherwise idle) DVE in f32
                    # — same precision as PSUM accumulation. The partition
                    # reduction then needs just ONE matmul per 128-q slice.
                    if kt == 0:
                        nc.vector.tensor_copy(acc[:], eslot[0])
                    else:
                        nc.vector.tensor_add(acc[:], acc[:], eslot[kt])

                # Transposed row-sums: acc-chunk as the stationary operand
                # puts q on the output partitions directly ([q,1] = chunk.T
                # @ ones). fp32 matmul at N=1 is ~LDW-cost only. Done before
                # PX so the reciprocals are long ready when O-proj needs
                # them.
                rcp = smlp.tile([128, NQB], f32, tag="rcp")
                for qs in range(QB // 128):
                    ps_s = psS.tile([128, 1], f32, tag="psS")
                    nc.tensor.matmul(
                        ps_s[:], acc[:, qs * 128:(qs + 1) * 128], ones_sb[:],
                        start=True, stop=True,
                    )
                    nc.vector.reciprocal(rcp[:, qs:qs + 1], ps_s[:])

                # PXT[d, q] = sum_k x[k, d] * EXP[k, q]: 24 bf16 k-slabs
                # plus 4 fp8 DoubleRow MMs covering the trailing 8 slabs
                # (two slabs per MM at bf16 per-MM cost).
                pxt_sb = pxp.tile([128, ND, QB], bf16, tag="pxt")
                for dt in range(ND):
                    ps_px = psX.tile([128, QB], f32, tag="psX")
                    for kt in range(NKB):
                        nc.tensor.matmul(
                            ps_px[:],
                            xr_sb[:, kt, dt * 128:(dt + 1) * 128],
                            eslot[kt],
                            start=(kt == 0), stop=False,
                        )
                    for i in range(NK8 // 2):
                        nc.tensor.matmul(
                            ps_px[:],
                            xr8_sb[:, 2 * i:2 * i + 2, dt * 128:(dt + 1) * 128],
                            exp8[:, 2 * i:2 * i + 2, :],
                            start=False, stop=(i == NK8 // 2 - 1),
                            perf_mode=DR,
                        )
                    nc.vector.tensor_copy(pxt_sb[:, dt, :], ps_px[:])

                for qs in range(QB // 128):
                    for ec in range(2):
                        ps_o = psQ.tile([128, 512], f32, tag="psQ")
                        for dt in range(ND):
                            nc.tensor.matmul(
                                ps_o[:],
                                pxt_sb[:, dt, qs * 128:(qs + 1) * 128],
                                wvt_sb[:, dt, ec * 512:(ec + 1) * 512],
                                start=(dt == 0), stop=(dt == ND - 1),
                            )
                        o_sb = stgp.tile([128, 512], bf16, tag="stg")
                        nc.scalar.mul(o_sb[:], ps_o[:], rcp[:, qs:qs + 1])
                        nc.gpsimd.dma_start(
                            out=out_d[q0 + qs * 128:q0 + qs * 128 + 128,
                                      ec * 512:(ec + 1) * 512],
                            in_=o_sb[:],
                        )
    nc.compile()
    return nc


def kernel(inp_emb, Wq, Wk, Wv):
    global LAST_RESULT
    from concourse.bass_utils import run_bass_kernel_spmd

    bf = ml_dtypes.bfloat16
    e4 = ml_dtypes.float8_e4m3
    x = np.asarray(inp_emb, dtype=np.float32)
    a = np.ascontiguousarray(
        np.asarray(Wq, np.float32).T @ np.asarray(Wk, np.float32)
    ).astype(bf)
    wvt = np.ascontiguousarray(np.asarray(Wv, np.float32).T).astype(bf)

    in_maps = []
    for c in range(8):
        b, h = divmod(c, 2)
        x_re = np.concatenate(
            [x[b, h * SQ:(h + 1) * SQ], x[b, (1 - h) * SQ:(2 - h) * SQ]], axis=0
        )
        xr = np.ascontiguousarray(x_re[:128 * NKB]).astype(bf)   # (3072, 1024)
        xr8 = np.ascontiguousarray(x_re[128 * NKB:]).astype(e4)  # (1024, 1024)
        xt = np.ascontiguousarray(x_re.T).astype(bf)             # (1024, 4096)
        in_maps.append({"xt": xt, "xr": xr, "xr8": xr8, "at": a, "wvt": wvt})

    if "nc" not in _CACHE:
        _CACHE["nc"] = _build()
    nc = _CACHE["nc"]

    res = run_bass_kernel_spmd(nc, in_maps, list(range(8)), trace=TRACE)
    LAST_RESULT = res

    out = np.empty((4, S, D), dtype=np.float32)
    for c in range(8):
        b, h = divmod(c, 2)
        out[b, h * SQ:(h + 1) * SQ] = res.results[c]["out"].astype(np.float32)
    return out
